# revision 30
# baseline (speedup 1.0000x reference)
"""Causal self-attention on 8 Trainium2 NeuronCores.

Sharding: tensor-parallel on heads (2 of 16 heads per core); row-parallel
output projection with fp16 partials summed on the host.

v2: mixed-precision fp8 acceleration on top of the v1 layout.
  - QKV projections for LATE tokens (t >= 512 within each batch element)
    run as fp8e4m3 DoubleRow matmuls (2 contraction chunks per matmul,
    2x PE throughput). Early tokens stay fp16, so the large-magnitude
    early out rows keep fp16 accuracy (causality confines fp8 error to
    late rows, where softmax averaging damps it).
  - PV for late-query jobs (j >= 1) runs entirely as fp8 DoubleRow
    chunk PAIRS (lhsT = v8 [128 keys, 2 chunks, 128] with a ones-column
    at 64 for the softmax-Z row and zero pad 65:128 — dual-fp8
    ldweights requires stationary M in {64,128}); 2x PE throughput.
    Early-query jobs (j=0) keep a pure-fp16 v16/ex16 path so early
    rows stay fp16-exact.
  - exp is split across the scalar engine (exact exp, fp8/fp16 out)
    and the DVE (single-pass Schraudolph bit-trick: qT/kT are pre-scaled
    by sqrt(log2 e) so the ST psum is s*8*log2e; one tensor_scalar
    (add, max) with a saturating-u8 convert produces the fp8e4m3 bit
    pattern of ~exp(s-4)). This removes the v1 scalar-engine exp
    bottleneck (~165us).
  - exp bias is -4 so exp(s_max~8.7 - 4) < 240 (fp8e4m3 max); smaller
    biases flush small p to fp8 subnormals/zero (z=0 NaN risk on
    singleton rows), so the fp16 j=0 path is also a correctness guard.
  - masks (tri-mul on diag exp tiles, fp16 for j=0 / fp8 for late jobs)
    run on gpsimd, which is SBUF-only (no PSUM access) but otherwise
    idle; the odd diag chunk's masked gap is pre-zeroed so each DR pair
    can stream the even chunk's full column range.
  - ST stays fp16: matmul cost is out-columns x cycles regardless of
    contraction, so K=64 gains nothing from fp8 (measured).
"""

import json

import numpy as np

import concourse.bass as bass
import concourse.mybir as mybir
import concourse.tile as tile
import concourse.bass2jax as bass2jax
import concourse.bass_utils as bass_utils
from concourse.bass import ts
from concourse.masks import make_identity, make_upper_triangular

B, T, C, H, D = 4, 2048, 1024, 16, 64
NCORES = 8
HL = H // NCORES          # heads per core = 2
HD = HL * D               # local head dims = 128
TF = B * T                # flattened tokens = 8192
NKC = C // 128            # contraction chunks for projections = 8
NTB = TF // 512           # 512-wide token blocks = 16
QB = 512                  # q block width
NQB = T // QB             # q blocks per batch elem = 4
TKC = T // 128            # 128-wide k chunks per batch elem = 16

f32 = mybir.dt.float32
f16 = mybir.dt.float16
f8 = mybir.dt.float8e4
u8 = mybir.dt.uint8
u16 = mybir.dt.uint16
EXP = mybir.ActivationFunctionType.Exp
IDENT = mybir.ActivationFunctionType.Identity
DR = mybir.MatmulPerfMode.DoubleRow

LOG2E = 1.4426950408889634
EXP_BIAS = -4.0           # exp(s - 4): keeps exp well inside fp8e4m3 (+-240)
GAMMA = float(np.sqrt(LOG2E))   # qT/kT prescale so st = s * 8*log2e
SC = float(1.0 / (8.0 * LOG2E))  # activation scale recovering s from st
# Schraudolph constants: u8 = round(max(st + C0U8, 0)) is the e4m3 bit
# pattern of ~exp(s + EXP_BIAS); u16 = round(st*128 + C0U16) for fp16.
C0U8 = float(7 * 8 + 8 * LOG2E * EXP_BIAS)
C0U16 = float(15 * 1024 + 1024 * LOG2E * EXP_BIAS)

NP16 = np.float16

# exp/ob engine routing patterns ('A' = scalar/ACT, 'D' = DVE)
ROUTE8 = ['A', 'D']       # non-diag fp8 exp chunks
ROUTE16 = ['A', 'D']      # diag fp16 exp chunks on late jobs
ROUTE_OB = ['D', 'A', 'D', 'A', 'D', 'A', 'D', 'A']


# --- workaround: this walrus build accepts at most one sync wait per
# instruction; Tile's final drain carries one wait per outstanding proc.
# Hoist surplus waits onto single-wait drain carriers in the BIR json.
_orig_compile_bir_kernel = None

MAX_WAITS_COMPUTE = 1
MAX_WAITS_CTRL = 1


def _split_waits_in_bir(bir_json):
    d = json.loads(bir_json)
    n = 0
    for f in d.get("functions", []):
        for bb in f.get("blocks", []):
            insts = bb.get("instructions", [])
            new_insts = []
            for inst in insts:
                si = inst.get("sync_info") or {}
                waits = si.get("on_wait") or []
                limit = (
                    MAX_WAITS_CTRL
                    if inst["opcode"]
                    in ("Drain", "EventSemaphore", "NoOp", "DMACopy", "DMA")
                    else MAX_WAITS_COMPUTE
                )
                if len(waits) > limit:
                    surplus = waits[:-limit]
                    for k, w in enumerate(surplus):
                        new_insts.append({
                            "name": f"{inst['name']}_wsplit{k}",
                            "engine": inst["engine"],
                            "opcode": "EventSemaphore",
                            "ins": [],
                            "outs": [],
                            "debug": inst.get("debug", 0),
                            "sync_info": {"on_update": [], "on_wait": [w]},
                        })
                        n += 1
                    si["on_wait"] = waits[-limit:]
                    inst["sync_info"] = si
                new_insts.append(inst)
            bb["instructions"] = new_insts
    return json.dumps(d).encode()


def _install_wait_split():
    global _orig_compile_bir_kernel
    if _orig_compile_bir_kernel is not None:
        return
    _orig_compile_bir_kernel = bass2jax.compile_bir_kernel

    def _patched(bir_json, tmpdir, neff_name="file.neff"):
        return _orig_compile_bir_kernel(
            _split_waits_in_bir(bir_json), tmpdir, neff_name
        )

    bass2jax.compile_bir_kernel = _patched


def build_program():
    nc = bass.Bass()
    xT16 = nc.declare_dram_parameter("xT16", [C, B * QB], f16, isOutput=False)
    xT8 = nc.declare_dram_parameter("xT8", [C, B * (T - QB)], f8, isOutput=False)
    wqkvT = nc.declare_dram_parameter("wqkvT", [C, 3 * HD], f16, isOutput=False)
    wqkvT8 = nc.declare_dram_parameter("wqkvT8", [C, 3 * HD], f8, isOutput=False)
    wpT = nc.declare_dram_parameter("wpT", [HD, C], f16, isOutput=False)
    bqkv = nc.declare_dram_parameter("bqkv", [HD, 3], f32, isOutput=False)
    outT = nc.declare_dram_parameter("outT", [C, TF], f16, isOutput=True)

    with tile.TileContext(nc) as tc:
        with (
            tc.tile_pool(name="consts", bufs=1) as consts,
            tc.tile_pool(name="persist", bufs=1) as persist,
        ):
            # weight loads first, split per contraction chunk across the
            # scalar/vector DGE queues so they parallelize and never perturb
            # the x-stream SP queue
            wq_sb = consts.tile([128, NKC, 3 * HD], f16)
            wqr = wqkvT.rearrange("(kc p) n -> p kc n", p=128)
            wq8_sb = consts.tile([128, NKC // 2, 2, 3 * HD], f8)
            wq8r = wqkvT8.rearrange("(kp i p) n -> p kp i n", i=2, p=128)
            # interleave fp16/fp8 weight loads so the first late block
            # (tb=1) is not gated behind all 8 fp16 chunks
            for kc in range(NKC):
                nc.scalar.dma_start(wq_sb[:, kc, :], wqr[:, kc, :])
                if kc < NKC // 2:
                    nc.scalar.dma_start(
                        wq8_sb[:, kc, :, :], wq8r[:, kc, :, :]
                    )
            wp_sb = consts.tile([HD, C], f16)
            nc.scalar.dma_start(wp_sb, wpT[:, :])
            b_sb = consts.tile([HD, 3], f32)
            nc.scalar.dma_start(b_sb, bqkv[:, :])
            ident16 = consts.tile([128, 128], f16)
            make_identity(nc, ident16)

            tri = consts.tile([128, 128], f16)
            make_upper_triangular(nc, tri, val=1.0, diag=True)
            tri8 = consts.tile([128, 128], f8)
            make_upper_triangular(nc, tri8, val=1.0, diag=True)
            # selector for the denominator broadcast (as v1)
            sel65 = consts.tile([65, 128], f16)
            nc.vector.memset(sel65, 0.0)
            nc.vector.memset(sel65[0:1, 0:64], 1.0)
            nc.vector.memset(sel65[64:65, 64:128], 1.0)
            lnt = consts.tile([65, 512], f32)
            nc.vector.memset(lnt, 0.0)
            rec65 = consts.tile([65, 512], f16)
            nc.vector.memset(rec65, 1.0)
            expbias = consts.tile([128, 1], f32)
            nc.vector.memset(expbias, EXP_BIAS)

            qT = persist.tile([128, TF], f16)
            kT = persist.tile([128, TF], f16)
            yT = persist.tile([128, TF], f16)
            # v8: all keys, [key, b, pair, parity, head, 128] with ones col
            # 64 (softmax Z row) and zero pad 65:128 (dual-fp8 ldweights
            # requires stationary M in {64,128})
            v8 = persist.tile([128, B, TKC // 2, 2, HL, 128], f8)
            for b_i in range(B):
                nc.gpsimd.memset(v8[:, b_i, :, :, :, 64:128], 0.0)
            nc.gpsimd.memset(v8[:, :, :, :, :, 64], 1.0)
            # v16: early keys only (first 4 chunks per batch elem), fp16
            v16 = persist.tile([128, B, 4, HL, 66], f16)
            for b_i in range(B):
                for kc_i in range(4):
                    nc.vector.memset(v16[:, b_i, kc_i, :, 64], 1.0)

            xT16r = xT16.rearrange("(kc p) t -> p kc t", p=128)
            xT8r = xT8.rearrange("(kp i p) t -> p kp i t", i=2, p=128)

            # ---- phase 1: QKV projections ----
            with (
                tc.tile_pool(name="p1", bufs=2) as p1,
                tc.tile_pool(name="ps1", bufs=4, space="PSUM") as ps1,
                tc.tile_pool(name="pst", bufs=2, space="PSUM") as pst,
            ):
                for tb in range(NTB):
                    b_i, jb = divmod(tb, NQB)
                    tsl = ts(tb, 512)
                    psq = ps1.tile([128, 512], f32, tag="qkvps")
                    psk = ps1.tile([128, 512], f32, tag="qkvps")
                    psv = ps1.tile([128, 512], f32, tag="qkvps")
                    pss = [psq, psk, psv]
                    if jb == 0:
                        # early tokens: fp16
                        ecol = ts(b_i, 512)
                        for kcp in range(NKC // 2):
                            xt = p1.tile([128, 2, 512], f16, tag="xt", bufs=10)
                            nc.sync.dma_start(
                                xt, xT16r[:, 2 * kcp:2 * kcp + 2, ecol]
                            )
                            for i in range(2):
                                kc = 2 * kcp + i
                                for pr in range(3):
                                    nc.tensor.matmul(
                                        pss[pr],
                                        lhsT=wq_sb[:, kc, ts(pr, HD)],
                                        rhs=xt[:, i, :],
                                        start=(kc == 0),
                                        stop=(kc == NKC - 1),
                                    )
                    else:
                        # late tokens: fp8 DoubleRow, 2 chunks per matmul
                        lcol = slice(
                            b_i * (T - QB) + (jb - 1) * 512,
                            b_i * (T - QB) + jb * 512,
                        )
                        for kcp in range(NKC // 2):
                            xt8 = p1.tile([128, 2, 512], f8, tag="xt8", bufs=12)
                            nc.sync.dma_start(xt8, xT8r[:, kcp, :, lcol])
                            for pr in range(3):
                                nc.tensor.matmul(
                                    pss[pr],
                                    lhsT=wq8_sb[:, kcp, :, ts(pr, HD)],
                                    rhs=xt8,
                                    start=(kcp == 0),
                                    stop=(kcp == NKC // 2 - 1),
                                    perf_mode=DR,
                                )
                    # q/k casts with the Schraudolph prescale on ACT
                    nc.scalar.activation(
                        qT[:, tsl], psq, IDENT, bias=b_sb[:, 0:1], scale=GAMMA
                    )
                    nc.scalar.activation(
                        kT[:, tsl], psk, IDENT, bias=b_sb[:, 1:2], scale=GAMMA
                    )
                    vt16 = p1.tile([128, 512], f16, tag="vt16")
                    nc.vector.tensor_scalar_add(vt16, psv, b_sb[:, 2:3])
                    pt16 = pst.tile([128, 4, 128], f16, tag="vtp16")
                    for i in range(4):
                        nc.tensor.transpose(
                            pt16[:, i, :], vt16[:, ts(i, 128)], ident16
                        )
                    # 4 chunks land as one [128, 2, 2, HL, 64] copy
                    bb, kc0 = divmod(tb * 4, TKC)
                    nc.vector.tensor_copy(
                        v8[:, bb, kc0 // 2:kc0 // 2 + 2, :, :, 0:64],
                        pt16.rearrange(
                            "p (a c) (h d) -> p a c h d", a=2, h=HL
                        ),
                    )
                    if jb == 0:
                        # early blocks additionally keep v in fp16
                        nc.vector.tensor_copy(
                            v16[:, b_i, :, :, 0:64],
                            pt16.rearrange("p a (h d) -> p a h d", h=HL),
                        )

            # ---- phase 2: causal attention + output projection ----
            rcount = [0, 0, 0]  # route counters: nondiag, diag, ob

            with (
                tc.tile_pool(name="p2", bufs=2) as p2,
                tc.tile_pool(name="ps2", bufs=1, space="PSUM") as ps2,
            ):
                def make_job(b_i, j):
                    q_off = b_i * T + j * QB
                    qsl = slice(q_off, q_off + QB)
                    nkc = 4 * (j + 1)
                    diag0 = nkc - 4
                    npr = diag0 // 2
                    ypq = [
                        ps2.tile([128, 512], f32, tag=f"y{h}", bufs=1,
                                 name=f"ypq{h}")
                        for h in range(HL)
                    ]
                    exs16 = {}
                    exs8 = {}

                    def stem(kc):
                        k_off = b_i * T + kc * 128
                        diag = kc >= diag0
                        r = kc * 128 - j * QB
                        lo = max(r, 0) if diag else 0
                        st = ps2.tile([128, 2, 512], f32, tag="st", bufs=2,
                                      name="st")
                        for h in range(HL):
                            nc.tensor.matmul(
                                st[:, h, lo:512],
                                lhsT=kT[ts(h, 64), k_off:k_off + 128],
                                rhs=qT[ts(h, 64), q_off + lo:q_off + QB],
                                start=True,
                                stop=True,
                            )
                        if j == 0:
                            # early-query jobs: fp16 exact path end-to-end
                            ex = p2.tile([128, HL, 512], f16, tag="ex16",
                                         bufs=4)
                            nc.scalar.activation(
                                ex[:, :, lo:512], st[:, :, lo:512], EXP,
                                scale=SC, bias=expbias,
                            )
                            for h in range(HL):
                                nc.gpsimd.tensor_mul(
                                    ex[:, h, r:r + 128], ex[:, h, r:r + 128],
                                    tri,
                                )
                            exs16[kc] = (ex, lo)
                            return
                        kp = kc // 2
                        par = kc % 2
                        if par == 0:
                            exs8[kp] = p2.tile([128, 2, HL, 512], f8,
                                               tag="ex8", bufs=8,
                                               name="ex8")
                            if diag:
                                # zero the masked gap of the odd chunk so
                                # the DR pair can stream the even chunk's
                                # full column range
                                nc.gpsimd.memset(
                                    exs8[kp][:, 1, :, lo:lo + 128], 0.0
                                )
                        ex8 = exs8[kp]
                        eng = ROUTE8[rcount[0] % len(ROUTE8)]
                        rcount[0] += 1
                        if eng == 'A':
                            nc.scalar.activation(
                                ex8[:, par, :, lo:512], st[:, :, lo:512],
                                EXP, scale=SC, bias=expbias,
                            )
                        else:
                            nc.vector.tensor_scalar(
                                ex8[:, par, :, lo:512].bitcast(u8),
                                st[:, :, lo:512], C0U8, 0.0,
                                op0=mybir.AluOpType.add,
                                op1=mybir.AluOpType.max,
                            )
                        if diag:
                            for h in range(HL):
                                nc.gpsimd.tensor_mul(
                                    ex8[:, par, h, r:r + 128],
                                    ex8[:, par, h, r:r + 128],
                                    tri8,
                                )

                    def pv_diag(kc):
                        # j == 0 only: fp16 PV with the ones-column v16
                        ex, lo = exs16.pop(kc)
                        for h in range(HL):
                            nc.tensor.matmul(
                                ypq[h][0:65, lo:512],
                                lhsT=v16[:, b_i, kc, h, 0:65],
                                rhs=ex[:, h, lo:512],
                                start=(kc == diag0),
                                stop=(kc == nkc - 1),
                            )

                    def pv_pair(kp, start=False, stop=False):
                        ex8 = exs8.pop(kp)
                        # diag pairs stream from the even chunk's lo
                        r0 = 2 * kp * 128 - j * QB
                        lo = max(r0, 0)
                        for h in range(HL):
                            nc.tensor.matmul(
                                ypq[h][:, lo:512],
                                lhsT=v8[:, b_i, kp, :, h, :],
                                rhs=ex8[:, :, h, lo:512],
                                start=start,
                                stop=stop,
                                perf_mode=DR,
                            )

                    def fin_scalar():
                        # softmax denominators: 1/x = exp(-ln(x)), Z in
                        # ypq row 64
                        for h in range(HL):
                            nc.scalar.activation(
                                lnt[64 * h:64 * h + 1, :], ypq[h][64:65, :],
                                mybir.ActivationFunctionType.Ln,
                            )
                        nc.scalar.activation(rec65, lnt, EXP, scale=-1.0)

                    yun = [None]

                    def fin_cast():
                        yun[0] = p2.tile([64, 2, 512], f16, tag="yun", bufs=2,
                                         name="yun")
                        for h in range(HL):
                            nc.vector.tensor_copy(
                                yun[0][:, h, :], ypq[h][0:64, :]
                            )

                    def fin_a():
                        bcrec = ps2.tile([128, 512], f32, tag="cp",
                                         bufs=2, name="bcrec")
                        nc.tensor.matmul(
                            bcrec, lhsT=sel65, rhs=rec65, start=True, stop=True
                        )
                        for h in range(HL):
                            nc.vector.tensor_mul(
                                yT[ts(h, 64), qsl], yun[0][:, h, :],
                                bcrec[ts(h, 64), :],
                            )

                    def fin_b_piece(op, last=False):
                        # one quarter of the row-parallel output projection
                        if last:
                            for i in range(2):
                                oc = 2 * op + i
                                pp = ps2.tile([128, 512], f32, tag="cp",
                                              bufs=2, name="pp")
                                nc.tensor.matmul(
                                    pp,
                                    lhsT=wp_sb[:, ts(oc, 128)],
                                    rhs=yT[:, qsl],
                                    start=True,
                                    stop=True,
                                )
                                obl = p2.tile([128, 512], f16, tag="obl",
                                              bufs=8, name="obl")
                                if oc % 2 == 0:
                                    nc.vector.tensor_copy(obl, pp)
                                else:
                                    nc.scalar.copy(obl, pp)
                                nc.sync.dma_start(
                                    outT[ts(oc, 128), qsl], obl
                                )
                            return
                        ob = p2.tile([128, 2, 512], f16, tag="ob", bufs=6)
                        for i in range(2):
                            oc = 2 * op + i
                            pp = ps2.tile([128, 512], f32, tag="cp",
                                          bufs=2, name="pp")
                            nc.tensor.matmul(
                                pp,
                                lhsT=wp_sb[:, ts(oc, 128)],
                                rhs=yT[:, qsl],
                                start=True,
                                stop=True,
                            )
                            eng = ROUTE_OB[rcount[2] % len(ROUTE_OB)]
                            rcount[2] += 1
                            if eng == 'D':
                                nc.vector.tensor_copy(ob[:, i, :], pp)
                            else:
                                nc.scalar.copy(ob[:, i, :], pp)
                        nc.sync.dma_start(
                            outT[op * 256:(op + 1) * 256, qsl].rearrange(
                                "(c p) t -> p c t", p=128
                            ),
                            ob,
                        )

                    def fin_b(last=False):
                        for op in range(4):
                            fin_b_piece(op, last=last)

                    return (nkc, diag0, npr, stem, pv_diag, pv_pair,
                            fin_scalar, fin_cast, fin_a, fin_b, fin_b_piece)

                # Emission: diag chunks first (their exp+mask chains get
                # maximum overlap), then non-diag chunks; all PVs for late
                # jobs are fp8 DoubleRow pairs.
                jobs = [(b_i, j) for b_i in range(B) for j in range(NQB)]
                prev_a = prev_b = None
                for b_i, j in jobs:
                    (nkc, diag0, npr, stem, pv_diag, pv_pair,
                     fin_scalar, fin_cast, fin_a, fin_b,
                     fin_b_piece) = make_job(b_i, j)
                    stem(diag0)
                    stem(diag0 + 1)
                    if prev_a is not None:
                        prev_a()
                    stem(diag0 + 2)
                    stem(diag0 + 3)
                    if j == 0:
                        for kc in range(4):
                            pv_diag(kc)
                    else:
                        dp = diag0 // 2
                        pend = [(dp, 0)] + [
                            (kp, 2 * kp + 3) for kp in range(dp)
                        ] + [(dp + 1, 10 ** 9)]
                        first, last = dp, dp + 1
                        for idx in range(diag0):
                            stem(idx)
                            if pend and idx >= pend[0][1]:
                                kp = pend.pop(0)[0]
                                pv_pair(kp, start=(kp == first),
                                        stop=(kp == last))
                        for kp, _ in pend:
                            pv_pair(kp, start=(kp == first),
                                    stop=(kp == last))
                    fin_scalar()
                    fin_cast()
                    if prev_b is not None:
                        prev_b()
                    prev_a, prev_b = fin_a, fin_b
                prev_a()
                prev_b(last=True)
    return nc


_program = None


def _get_program():
    global _program
    if _program is None:
        _install_wait_split()
        _program = build_program()
    return _program


def kernel(x, Wq, bq, Wk, bk, Wv, bv, Wp, bp):
    nc = _get_program()

    x = np.asarray(x, dtype=np.float32)
    xr = x.reshape(B, T, C)
    x_early = np.ascontiguousarray(
        xr[:, :QB].reshape(B * QB, C).T.astype(NP16)
    )
    import ml_dtypes
    x_late = np.ascontiguousarray(
        xr[:, QB:].reshape(B * (T - QB), C).T.astype(ml_dtypes.float8_e4m3)
    )
    in_maps = []
    for core in range(NCORES):
        rows = slice(core * HD, (core + 1) * HD)
        wqkv_f32 = np.concatenate(
            [np.asarray(W, np.float32)[rows].T for W in (Wq, Wk, Wv)], axis=1
        )
        wqkvT = np.ascontiguousarray(wqkv_f32.astype(NP16))
        wqkvT8 = np.ascontiguousarray(wqkv_f32.astype(ml_dtypes.float8_e4m3))
        wpT = np.ascontiguousarray(
            np.asarray(Wp, np.float32)[:, rows].T.astype(NP16)
        )
        bq_l = np.stack(
            [
                np.asarray(bq, np.float32)[rows] * GAMMA,
                np.asarray(bk, np.float32)[rows] * GAMMA,
                np.asarray(bv, np.float32)[rows],
            ],
            axis=1,
        )
        in_maps.append(
            {
                "xT16": x_early,
                "xT8": x_late,
                "wqkvT": wqkvT,
                "wqkvT8": wqkvT8,
                "wpT": wpT,
                "bqkv": np.ascontiguousarray(bq_l),
            }
        )

    r = bass_utils.run_bass_kernel_spmd(nc, in_maps, list(range(NCORES)))
    acc = r.results[0]["outT"].astype(np.float32)
    for core in range(1, NCORES):
        acc = acc + r.results[core]["outT"].astype(np.float32)
    out = acc.T.reshape(B, T, C) + np.asarray(bp, np.float32)[None, None, :]
    return out.astype(np.float32)


# revision 31
# speedup vs baseline: 1.0182x; 1.0182x over previous
"""Causal self-attention on 8 Trainium2 NeuronCores.

Sharding: tensor-parallel on heads (2 of 16 heads per core); row-parallel
output projection with fp16 partials summed on the host.

v2: mixed-precision fp8 acceleration on top of the v1 layout.
  - QKV projections for LATE tokens (t >= 512 within each batch element)
    run as fp8e4m3 DoubleRow matmuls (2 contraction chunks per matmul,
    2x PE throughput). Early tokens stay fp16, so the large-magnitude
    early out rows keep fp16 accuracy (causality confines fp8 error to
    late rows, where softmax averaging damps it).
  - PV for late-query jobs (j >= 1) runs entirely as fp8 DoubleRow
    chunk PAIRS (lhsT = v8 [128 keys, 2 chunks, 128] with a ones-column
    at 64 for the softmax-Z row and zero pad 65:128 — dual-fp8
    ldweights requires stationary M in {64,128}); 2x PE throughput.
    Early-query jobs (j=0) keep a pure-fp16 v16/ex16 path so early
    rows stay fp16-exact.
  - exp is split across the scalar engine (exact exp, fp8/fp16 out)
    and the DVE (single-pass Schraudolph bit-trick: qT/kT are pre-scaled
    by sqrt(log2 e) so the ST psum is s*8*log2e; one tensor_scalar
    (add, max) with a saturating-u8 convert produces the fp8e4m3 bit
    pattern of ~exp(s-4)). This removes the v1 scalar-engine exp
    bottleneck (~165us).
  - exp bias is -4 so exp(s_max~8.7 - 4) < 240 (fp8e4m3 max); smaller
    biases flush small p to fp8 subnormals/zero (z=0 NaN risk on
    singleton rows), so the fp16 j=0 path is also a correctness guard.
  - masks (tri-mul on diag exp tiles, fp16 for j=0 / fp8 for late jobs)
    run on gpsimd, which is SBUF-only (no PSUM access) but otherwise
    idle; the odd diag chunk's masked gap is pre-zeroed so each DR pair
    can stream the even chunk's full column range.
  - ST stays fp16: matmul cost is out-columns x cycles regardless of
    contraction, so K=64 gains nothing from fp8 (measured).
"""

import json

import numpy as np

import concourse.bass as bass
import concourse.mybir as mybir
import concourse.tile as tile
import concourse.bass2jax as bass2jax
import concourse.bass_utils as bass_utils
from concourse.bass import ts
from concourse.masks import make_identity, make_upper_triangular

B, T, C, H, D = 4, 2048, 1024, 16, 64
NCORES = 8
HL = H // NCORES          # heads per core = 2
HD = HL * D               # local head dims = 128
TF = B * T                # flattened tokens = 8192
NKC = C // 128            # contraction chunks for projections = 8
NTB = TF // 512           # 512-wide token blocks = 16
QB = 512                  # q block width
NQB = T // QB             # q blocks per batch elem = 4
TKC = T // 128            # 128-wide k chunks per batch elem = 16

f32 = mybir.dt.float32
f16 = mybir.dt.float16
f8 = mybir.dt.float8e4
u8 = mybir.dt.uint8
u16 = mybir.dt.uint16
EXP = mybir.ActivationFunctionType.Exp
IDENT = mybir.ActivationFunctionType.Identity
DR = mybir.MatmulPerfMode.DoubleRow

LOG2E = 1.4426950408889634
EXP_BIAS = -4.0           # exp(s - 4): keeps exp well inside fp8e4m3 (+-240)
GAMMA = float(np.sqrt(LOG2E))   # qT/kT prescale so st = s * 8*log2e
SC = float(1.0 / (8.0 * LOG2E))  # activation scale recovering s from st
# Schraudolph constants: u8 = round(max(st + C0U8, 0)) is the e4m3 bit
# pattern of ~exp(s + EXP_BIAS); u16 = round(st*128 + C0U16) for fp16.
C0U8 = float(7 * 8 + 8 * LOG2E * EXP_BIAS)
C0U16 = float(15 * 1024 + 1024 * LOG2E * EXP_BIAS)

NP16 = np.float16

# exp/ob engine routing patterns ('A' = scalar/ACT, 'D' = DVE)
ROUTE8 = ['A', 'D']       # non-diag fp8 exp chunks
ROUTE16 = ['A', 'D']      # diag fp16 exp chunks on late jobs
ROUTE_OB = ['D', 'A', 'D', 'A', 'D', 'A', 'D', 'A']


# --- workaround: this walrus build accepts at most one sync wait per
# instruction; Tile's final drain carries one wait per outstanding proc.
# Hoist surplus waits onto single-wait drain carriers in the BIR json.
_orig_compile_bir_kernel = None

MAX_WAITS_COMPUTE = 1
MAX_WAITS_CTRL = 1


def _split_waits_in_bir(bir_json):
    d = json.loads(bir_json)
    n = 0
    for f in d.get("functions", []):
        for bb in f.get("blocks", []):
            insts = bb.get("instructions", [])
            new_insts = []
            for inst in insts:
                si = inst.get("sync_info") or {}
                waits = si.get("on_wait") or []
                limit = (
                    MAX_WAITS_CTRL
                    if inst["opcode"]
                    in ("Drain", "EventSemaphore", "NoOp", "DMACopy", "DMA")
                    else MAX_WAITS_COMPUTE
                )
                if len(waits) > limit:
                    surplus = waits[:-limit]
                    for k, w in enumerate(surplus):
                        new_insts.append({
                            "name": f"{inst['name']}_wsplit{k}",
                            "engine": inst["engine"],
                            "opcode": "EventSemaphore",
                            "ins": [],
                            "outs": [],
                            "debug": inst.get("debug", 0),
                            "sync_info": {"on_update": [], "on_wait": [w]},
                        })
                        n += 1
                    si["on_wait"] = waits[-limit:]
                    inst["sync_info"] = si
                new_insts.append(inst)
            bb["instructions"] = new_insts
    return json.dumps(d).encode()


def _install_wait_split():
    global _orig_compile_bir_kernel
    if _orig_compile_bir_kernel is not None:
        return
    _orig_compile_bir_kernel = bass2jax.compile_bir_kernel

    def _patched(bir_json, tmpdir, neff_name="file.neff"):
        return _orig_compile_bir_kernel(
            _split_waits_in_bir(bir_json), tmpdir, neff_name
        )

    bass2jax.compile_bir_kernel = _patched


def build_program():
    nc = bass.Bass()
    xT16 = nc.declare_dram_parameter("xT16", [C, B * QB], f16, isOutput=False)
    xT8 = nc.declare_dram_parameter("xT8", [C, B * (T - QB)], f8, isOutput=False)
    wqkvT = nc.declare_dram_parameter("wqkvT", [C, 3 * HD], f16, isOutput=False)
    wqkvT8 = nc.declare_dram_parameter("wqkvT8", [C, 3 * HD], f8, isOutput=False)
    wpT = nc.declare_dram_parameter("wpT", [HD, C], f16, isOutput=False)
    bqkv = nc.declare_dram_parameter("bqkv", [HD, 3], f32, isOutput=False)
    outT = nc.declare_dram_parameter("outT", [C, TF], f16, isOutput=True)

    with tile.TileContext(nc) as tc:
        with (
            tc.tile_pool(name="consts", bufs=1) as consts,
            tc.tile_pool(name="persist", bufs=1) as persist,
        ):
            # weight loads first, split per contraction chunk across the
            # scalar/vector DGE queues so they parallelize and never perturb
            # the x-stream SP queue
            wq_sb = consts.tile([128, NKC, 3 * HD], f16)
            wqr = wqkvT.rearrange("(kc p) n -> p kc n", p=128)
            wq8_sb = consts.tile([128, NKC // 2, 2, 3 * HD], f8)
            wq8r = wqkvT8.rearrange("(kp i p) n -> p kp i n", i=2, p=128)
            # interleave fp16/fp8 weight loads so the first late block
            # (tb=1) is not gated behind all 8 fp16 chunks
            for kc in range(NKC):
                nc.scalar.dma_start(wq_sb[:, kc, :], wqr[:, kc, :])
                if kc < NKC // 2:
                    nc.scalar.dma_start(
                        wq8_sb[:, kc, :, :], wq8r[:, kc, :, :]
                    )
            wp_sb = consts.tile([HD, C], f16)
            nc.scalar.dma_start(wp_sb, wpT[:, :])
            b_sb = consts.tile([HD, 3], f32)
            nc.scalar.dma_start(b_sb, bqkv[:, :])
            ident16 = consts.tile([128, 128], f16)
            make_identity(nc, ident16)

            tri = consts.tile([128, 128], f16)
            make_upper_triangular(nc, tri, val=1.0, diag=True)
            tri8 = consts.tile([128, 128], f8)
            make_upper_triangular(nc, tri8, val=1.0, diag=True)
            # selector for the denominator broadcast (as v1)
            sel65 = consts.tile([65, 128], f16)
            nc.vector.memset(sel65, 0.0)
            nc.vector.memset(sel65[0:1, 0:64], 1.0)
            nc.vector.memset(sel65[64:65, 64:128], 1.0)
            lnt = consts.tile([65, 512], f32)
            nc.vector.memset(lnt, 0.0)
            rec65 = consts.tile([65, 512], f16)
            nc.vector.memset(rec65, 1.0)
            expbias = consts.tile([128, 1], f32)
            nc.vector.memset(expbias, EXP_BIAS)

            qT = persist.tile([128, TF], f16)
            kT = persist.tile([128, TF], f16)
            yT = persist.tile([128, TF], f16)
            # v8: all keys, [key, b, pair, parity, head, 128] with ones col
            # 64 (softmax Z row) and zero pad 65:128 (dual-fp8 ldweights
            # requires stationary M in {64,128})
            v8 = persist.tile([128, B, TKC // 2, 2, HL, 128], f8)
            for b_i in range(B):
                nc.gpsimd.memset(v8[:, b_i, :, :, :, 64:128], 0.0)
            nc.gpsimd.memset(v8[:, :, :, :, :, 64], 1.0)
            # v16: early keys only (first 4 chunks per batch elem), fp16
            v16 = persist.tile([128, B, 4, HL, 66], f16)
            for b_i in range(B):
                for kc_i in range(4):
                    nc.vector.memset(v16[:, b_i, kc_i, :, 64], 1.0)

            xT16r = xT16.rearrange("(kc p) t -> p kc t", p=128)
            xT8r = xT8.rearrange("(kp i p) t -> p kp i t", i=2, p=128)

            # ---- phase 1: QKV projections ----
            with (
                tc.tile_pool(name="p1", bufs=2) as p1,
                tc.tile_pool(name="ps1", bufs=4, space="PSUM") as ps1,
                tc.tile_pool(name="pst", bufs=2, space="PSUM") as pst,
            ):
                for tb in range(NTB):
                    b_i, jb = divmod(tb, NQB)
                    tsl = ts(tb, 512)
                    psq = ps1.tile([128, 512], f32, tag="qkvps", bufs=6)
                    psk = ps1.tile([128, 512], f32, tag="qkvps", bufs=6)
                    psv = ps1.tile([128, 512], f32, tag="qkvps", bufs=6)
                    pss = [psq, psk, psv]
                    if jb == 0:
                        # early tokens: fp16
                        ecol = ts(b_i, 512)
                        for kcp in range(NKC // 2):
                            xt = p1.tile([128, 2, 512], f16, tag="xt", bufs=10)
                            nc.sync.dma_start(
                                xt, xT16r[:, 2 * kcp:2 * kcp + 2, ecol]
                            )
                            for i in range(2):
                                kc = 2 * kcp + i
                                for pr in range(3):
                                    nc.tensor.matmul(
                                        pss[pr],
                                        lhsT=wq_sb[:, kc, ts(pr, HD)],
                                        rhs=xt[:, i, :],
                                        start=(kc == 0),
                                        stop=(kc == NKC - 1),
                                    )
                    else:
                        # late tokens: fp8 DoubleRow, 2 chunks per matmul
                        lcol = slice(
                            b_i * (T - QB) + (jb - 1) * 512,
                            b_i * (T - QB) + jb * 512,
                        )
                        for kcp in range(NKC // 2):
                            xt8 = p1.tile([128, 2, 512], f8, tag="xt8", bufs=12)
                            nc.sync.dma_start(xt8, xT8r[:, kcp, :, lcol])
                            for pr in range(3):
                                nc.tensor.matmul(
                                    pss[pr],
                                    lhsT=wq8_sb[:, kcp, :, ts(pr, HD)],
                                    rhs=xt8,
                                    start=(kcp == 0),
                                    stop=(kcp == NKC // 2 - 1),
                                    perf_mode=DR,
                                )
                    # q/k casts with the Schraudolph prescale on ACT
                    nc.scalar.activation(
                        qT[:, tsl], psq, IDENT, bias=b_sb[:, 0:1], scale=GAMMA
                    )
                    nc.scalar.activation(
                        kT[:, tsl], psk, IDENT, bias=b_sb[:, 1:2], scale=GAMMA
                    )
                    vt16 = p1.tile([128, 512], f16, tag="vt16")
                    nc.vector.tensor_scalar_add(vt16, psv, b_sb[:, 2:3])
                    pt16 = pst.tile([128, 4, 128], f16, tag="vtp16")
                    for i in range(4):
                        nc.tensor.transpose(
                            pt16[:, i, :], vt16[:, ts(i, 128)], ident16
                        )
                    # 4 chunks land as one [128, 2, 2, HL, 64] copy
                    bb, kc0 = divmod(tb * 4, TKC)
                    nc.vector.tensor_copy(
                        v8[:, bb, kc0 // 2:kc0 // 2 + 2, :, :, 0:64],
                        pt16.rearrange(
                            "p (a c) (h d) -> p a c h d", a=2, h=HL
                        ),
                    )
                    if jb == 0:
                        # early blocks additionally keep v in fp16
                        nc.vector.tensor_copy(
                            v16[:, b_i, :, :, 0:64],
                            pt16.rearrange("p a (h d) -> p a h d", h=HL),
                        )

            # ---- phase 2: causal attention + output projection ----
            rcount = [0, 0, 0]  # route counters: nondiag, diag, ob

            with (
                tc.tile_pool(name="p2", bufs=2) as p2,
                tc.tile_pool(name="ps2", bufs=1, space="PSUM") as ps2,
            ):
                def make_job(b_i, j):
                    q_off = b_i * T + j * QB
                    qsl = slice(q_off, q_off + QB)
                    nkc = 4 * (j + 1)
                    diag0 = nkc - 4
                    npr = diag0 // 2
                    ypq = [
                        ps2.tile([128, 512], f32, tag=f"y{h}", bufs=1,
                                 name=f"ypq{h}")
                        for h in range(HL)
                    ]
                    exs16 = {}
                    exs8 = {}

                    def stem(kc):
                        k_off = b_i * T + kc * 128
                        diag = kc >= diag0
                        r = kc * 128 - j * QB
                        lo = max(r, 0) if diag else 0
                        st = ps2.tile([128, 2, 512], f32, tag="st", bufs=2,
                                      name="st")
                        for h in range(HL):
                            nc.tensor.matmul(
                                st[:, h, lo:512],
                                lhsT=kT[ts(h, 64), k_off:k_off + 128],
                                rhs=qT[ts(h, 64), q_off + lo:q_off + QB],
                                start=True,
                                stop=True,
                            )
                        if j == 0:
                            # early-query jobs: fp16 exact path end-to-end
                            ex = p2.tile([128, HL, 512], f16, tag="ex16",
                                         bufs=4)
                            nc.scalar.activation(
                                ex[:, :, lo:512], st[:, :, lo:512], EXP,
                                scale=SC, bias=expbias,
                            )
                            for h in range(HL):
                                nc.gpsimd.tensor_mul(
                                    ex[:, h, r:r + 128], ex[:, h, r:r + 128],
                                    tri,
                                )
                            exs16[kc] = (ex, lo)
                            return
                        kp = kc // 2
                        par = kc % 2
                        if par == 0:
                            exs8[kp] = p2.tile([128, 2, HL, 512], f8,
                                               tag="ex8", bufs=8,
                                               name="ex8")
                            if diag:
                                # zero the masked gap of the odd chunk so
                                # the DR pair can stream the even chunk's
                                # full column range
                                nc.gpsimd.memset(
                                    exs8[kp][:, 1, :, lo:lo + 128], 0.0
                                )
                        ex8 = exs8[kp]
                        eng = ROUTE8[rcount[0] % len(ROUTE8)]
                        rcount[0] += 1
                        if eng == 'A':
                            nc.scalar.activation(
                                ex8[:, par, :, lo:512], st[:, :, lo:512],
                                EXP, scale=SC, bias=expbias,
                            )
                        else:
                            nc.vector.tensor_scalar(
                                ex8[:, par, :, lo:512].bitcast(u8),
                                st[:, :, lo:512], C0U8, 0.0,
                                op0=mybir.AluOpType.add,
                                op1=mybir.AluOpType.max,
                            )
                        if diag:
                            for h in range(HL):
                                nc.gpsimd.tensor_mul(
                                    ex8[:, par, h, r:r + 128],
                                    ex8[:, par, h, r:r + 128],
                                    tri8,
                                )

                    def pv_diag(kc):
                        # j == 0 only: fp16 PV with the ones-column v16
                        ex, lo = exs16.pop(kc)
                        for h in range(HL):
                            nc.tensor.matmul(
                                ypq[h][0:65, lo:512],
                                lhsT=v16[:, b_i, kc, h, 0:65],
                                rhs=ex[:, h, lo:512],
                                start=(kc == diag0),
                                stop=(kc == nkc - 1),
                            )

                    def pv_pair(kp, start=False, stop=False):
                        ex8 = exs8.pop(kp)
                        # diag pairs stream from the even chunk's lo
                        r0 = 2 * kp * 128 - j * QB
                        lo = max(r0, 0)
                        for h in range(HL):
                            nc.tensor.matmul(
                                ypq[h][:, lo:512],
                                lhsT=v8[:, b_i, kp, :, h, :],
                                rhs=ex8[:, :, h, lo:512],
                                start=start,
                                stop=stop,
                                perf_mode=DR,
                            )

                    def fin_scalar():
                        # softmax denominators: 1/x = exp(-ln(x)), Z in
                        # ypq row 64
                        for h in range(HL):
                            nc.scalar.activation(
                                lnt[64 * h:64 * h + 1, :], ypq[h][64:65, :],
                                mybir.ActivationFunctionType.Ln,
                            )
                        nc.scalar.activation(rec65, lnt, EXP, scale=-1.0)

                    yun = [None]

                    def fin_cast():
                        yun[0] = p2.tile([64, 2, 512], f16, tag="yun", bufs=2,
                                         name="yun")
                        for h in range(HL):
                            nc.vector.tensor_copy(
                                yun[0][:, h, :], ypq[h][0:64, :]
                            )

                    def fin_a():
                        bcrec = ps2.tile([128, 512], f32, tag="cp",
                                         bufs=2, name="bcrec")
                        nc.tensor.matmul(
                            bcrec, lhsT=sel65, rhs=rec65, start=True, stop=True
                        )
                        for h in range(HL):
                            nc.vector.tensor_mul(
                                yT[ts(h, 64), qsl], yun[0][:, h, :],
                                bcrec[ts(h, 64), :],
                            )

                    def fin_b_piece(op, last=False):
                        # one quarter of the row-parallel output projection
                        if last:
                            for i in range(2):
                                oc = 2 * op + i
                                pp = ps2.tile([128, 512], f32, tag="cp",
                                              bufs=2, name="pp")
                                nc.tensor.matmul(
                                    pp,
                                    lhsT=wp_sb[:, ts(oc, 128)],
                                    rhs=yT[:, qsl],
                                    start=True,
                                    stop=True,
                                )
                                obl = p2.tile([128, 512], f16, tag="obl",
                                              bufs=8, name="obl")
                                if oc % 2 == 0:
                                    nc.vector.tensor_copy(obl, pp)
                                else:
                                    nc.scalar.copy(obl, pp)
                                nc.sync.dma_start(
                                    outT[ts(oc, 128), qsl], obl
                                )
                            return
                        ob = p2.tile([128, 2, 512], f16, tag="ob", bufs=6)
                        for i in range(2):
                            oc = 2 * op + i
                            pp = ps2.tile([128, 512], f32, tag="cp",
                                          bufs=2, name="pp")
                            nc.tensor.matmul(
                                pp,
                                lhsT=wp_sb[:, ts(oc, 128)],
                                rhs=yT[:, qsl],
                                start=True,
                                stop=True,
                            )
                            eng = ROUTE_OB[rcount[2] % len(ROUTE_OB)]
                            rcount[2] += 1
                            if eng == 'D':
                                nc.vector.tensor_copy(ob[:, i, :], pp)
                            else:
                                nc.scalar.copy(ob[:, i, :], pp)
                        nc.sync.dma_start(
                            outT[op * 256:(op + 1) * 256, qsl].rearrange(
                                "(c p) t -> p c t", p=128
                            ),
                            ob,
                        )

                    def fin_b(last=False):
                        for op in range(4):
                            fin_b_piece(op, last=last)

                    return (nkc, diag0, npr, stem, pv_diag, pv_pair,
                            fin_scalar, fin_cast, fin_a, fin_b, fin_b_piece)

                # Emission: diag chunks first (their exp+mask chains get
                # maximum overlap), then non-diag chunks; all PVs for late
                # jobs are fp8 DoubleRow pairs.
                jobs = [(b_i, j) for b_i in range(B) for j in range(NQB)]
                prev_a = prev_b = None
                for b_i, j in jobs:
                    (nkc, diag0, npr, stem, pv_diag, pv_pair,
                     fin_scalar, fin_cast, fin_a, fin_b,
                     fin_b_piece) = make_job(b_i, j)
                    stem(diag0)
                    stem(diag0 + 1)
                    if prev_a is not None:
                        prev_a()
                    stem(diag0 + 2)
                    stem(diag0 + 3)
                    if j == 0:
                        for kc in range(4):
                            pv_diag(kc)
                    else:
                        dp = diag0 // 2
                        pend = [(dp, 0)] + [
                            (kp, 2 * kp + 3) for kp in range(dp)
                        ] + [(dp + 1, 10 ** 9)]
                        first, last = dp, dp + 1
                        for idx in range(diag0):
                            stem(idx)
                            if pend and idx >= pend[0][1]:
                                kp = pend.pop(0)[0]
                                pv_pair(kp, start=(kp == first),
                                        stop=(kp == last))
                        for kp, _ in pend:
                            pv_pair(kp, start=(kp == first),
                                    stop=(kp == last))
                    fin_scalar()
                    fin_cast()
                    if prev_b is not None:
                        prev_b()
                    prev_a, prev_b = fin_a, fin_b
                prev_a()
                prev_b(last=True)
    return nc


_program = None


def _get_program():
    global _program
    if _program is None:
        _install_wait_split()
        _program = build_program()
    return _program


def kernel(x, Wq, bq, Wk, bk, Wv, bv, Wp, bp):
    nc = _get_program()

    x = np.asarray(x, dtype=np.float32)
    xr = x.reshape(B, T, C)
    x_early = np.ascontiguousarray(
        xr[:, :QB].reshape(B * QB, C).T.astype(NP16)
    )
    import ml_dtypes
    x_late = np.ascontiguousarray(
        xr[:, QB:].reshape(B * (T - QB), C).T.astype(ml_dtypes.float8_e4m3)
    )
    in_maps = []
    for core in range(NCORES):
        rows = slice(core * HD, (core + 1) * HD)
        wqkv_f32 = np.concatenate(
            [np.asarray(W, np.float32)[rows].T for W in (Wq, Wk, Wv)], axis=1
        )
        wqkvT = np.ascontiguousarray(wqkv_f32.astype(NP16))
        wqkvT8 = np.ascontiguousarray(wqkv_f32.astype(ml_dtypes.float8_e4m3))
        wpT = np.ascontiguousarray(
            np.asarray(Wp, np.float32)[:, rows].T.astype(NP16)
        )
        bq_l = np.stack(
            [
                np.asarray(bq, np.float32)[rows] * GAMMA,
                np.asarray(bk, np.float32)[rows] * GAMMA,
                np.asarray(bv, np.float32)[rows],
            ],
            axis=1,
        )
        in_maps.append(
            {
                "xT16": x_early,
                "xT8": x_late,
                "wqkvT": wqkvT,
                "wqkvT8": wqkvT8,
                "wpT": wpT,
                "bqkv": np.ascontiguousarray(bq_l),
            }
        )

    r = bass_utils.run_bass_kernel_spmd(nc, in_maps, list(range(NCORES)))
    acc = r.results[0]["outT"].astype(np.float32)
    for core in range(1, NCORES):
        acc = acc + r.results[core]["outT"].astype(np.float32)
    out = acc.T.reshape(B, T, C) + np.asarray(bp, np.float32)[None, None, :]
    return out.astype(np.float32)
